# revision 73
# baseline (speedup 1.0000x reference)
"""Trainium2 Bass kernel for nn_GaussianBasis (2D gaussian-splat sum rasterizer).

Math: out[c,d,h,w] = sum_n opacity_n * exp(-sigma_n(h,w)) * features[c,n,d]
where sigma is a per-gaussian quadratic form in pixel coords.

Strategy:
  - Gaussians have tiny support; bin them host-side into 16x16-pixel buckets
    with a sigma <= SIG_CUT cutoff ellipse (exp(-8) ~ 3.4e-4; dropped tail
    contributions stay far below the 2e-2 relative tolerance vs absmax ~2.7).
  - Each core owns a 32-row band = 2x16 buckets. Buckets are packed into
    128-partition "tiles" (one gaussian = one partition row, sum k <= 128,
    <= 5 buckets so each tile needs exactly 2 channel-units; the 2 smallest
    buckets form a final 1-unit "runt" tile so the tail transfer is tiny).
    sigma over a tile is ONE K=12 fp16 matmul against the shared
    bucket-centered phi = [x^2,y^2,xy,x,y,1] basis (hi/lo fp16 coefficient
    split for fp32-grade accuracy) -> [128 gauss, 256 px] PSUM. Packing cuts
    tiles from 16 to 7, cutting the serial exp chain ~2.3x.
  - ACT computes g = exp(-sigma) PSUM->SBUF fp16 in 2-tile groups; the first
    group is a single tile so the downstream pipeline primes one sigma
    earlier. Group g+1's sigma matmuls are emitted before group g's feature
    matmuls (software pipelining) so the exp chain never waits on PE.
  - Feature einsum: per tile, the buckets' 48 output channels form one dense
    column stream chopped into 128-column "units" (a bucket's channels may
    split across units): one fp16 matmul per unit with a [128,128] block
    weight -> PSUM [128, 256] with EVERY partition carrying payload - the
    information-theoretic floor for PSUM-evacuation volume (13 units/core).
  - PSUM is only readable by ACT/DVE (no DMA, no Pool). Per-tile fp32->fp16
    copies into one contiguous SBUF staging buffer: DVE inline for early
    tiles (it idles while ACT runs exps), ACT for the last NDEFER tiles,
    emitted after its final exp so the exp chain is never stretched.
  - Per-tile output DMAs (fp16, runs >= 512B: full bandwidth) overlap
    compute, alternating the SP-HWDGE and Pool-SWDGE issue pipes; the
    deferred tiles share one merged DMA plus a small final one. Host
    upcasts and reassembles the strips into [C,3,H,W].
  - No collectives: pixel-sharding keeps outputs disjoint.
"""

import sys

sys.path.insert(0, "/opt/trn_rl_repo")

import numpy as np
from contextlib import ExitStack

N, C, H, W = 2048, 16, 256, 256
NCORES = 8
BK = 16                      # bucket edge in pixels
PXB = BK * BK                # 256 px per bucket
NBR, NBC = (H // NCORES) // BK, W // BK   # 2 bucket-rows, 16 cols per core
CH = C * 3                   # 48 output channels
SIG_CUT = 8.0                # exp(-8) ~ 3.4e-4: negligible vs tolerance

_cached = {}


def _params(xyz_raw, cholesky_raw, features, opacity):
    """Per-gaussian params (fp64 host): centers, quadratic coeffs, cutoff
    radii, opacity-folded features."""
    xy = np.tanh(xyz_raw.astype(np.float64))
    cx = 0.5 * (xy[:, 0] + 1.0) * W
    cy = 0.5 * (xy[:, 1] + 1.0) * H
    chol = cholesky_raw.astype(np.float64) + np.array([0.5, 0.0, 0.5])
    l1, l2, l3 = chol[:, 0], chol[:, 1], chol[:, 2]
    a = l1 * l1
    b = l1 * l2
    c = l2 * l2 + l3 * l3
    det = a * c - b * b
    Aq, Bq, Cq = 0.5 * (c / det), -b / det, 0.5 * (a / det)
    rx = np.sqrt(2.0 * SIG_CUT * a) + 1.0
    ry = np.sqrt(2.0 * SIG_CUT * c) + 1.0
    featw = features.astype(np.float64) * opacity[:, 0][None, :, None]
    featw = np.transpose(featw, (1, 0, 2)).reshape(N, CH)
    return cx, cy, Aq, Bq, Cq, rx, ry, featw


def _pack(cx, cy, rx, ry):
    """Bin gaussians into per-core 16x16 buckets; first-fit-decreasing pack
    whole buckets into 128-partition tiles (oversized buckets split).
    Returns tiles[core] = list of tiles; tile = [(r, c, idx, offset), ...]
    sorted so tiles with more channel-units come first (aligns padding)."""
    h_lo = np.floor(cy - ry).astype(int)
    h_hi = np.ceil(cy + ry).astype(int)
    w_lo = np.floor(cx - rx).astype(int)
    w_hi = np.ceil(cx + rx).astype(int)
    nrow, ncol = H // BK, W // BK
    buckets = [[[] for _ in range(ncol)] for _ in range(nrow)]
    for n in range(N):
        for bh in range(max(0, h_lo[n] // BK), min(nrow, h_hi[n] // BK + 1)):
            for bw in range(max(0, w_lo[n] // BK), min(ncol, w_hi[n] // BK + 1)):
                buckets[bh][bw].append(n)

    tiles_per_core = []
    for core in range(NCORES):
        blist = []
        for r in range(NBR):
            for c in range(NBC):
                ns = np.asarray(buckets[core * NBR + r][c], dtype=int)
                for s in range(0, len(ns), 128):
                    blist.append((r, c, ns[s:s + 128]))
        blist.sort(key=lambda b: -len(b[2]))
        # cap buckets/tile at 5 so every tile needs exactly 2 channel-units
        # (48*5=240 <= 256 psum cols): uniform tiles, no oversized PSUM.
        # The 2 smallest buckets are reserved as a 1-unit "runt" tile that
        # goes LAST: the tail copy+DMA is then half-sized.
        runt = blist[-2:] if len(blist) > 2 else []
        main = blist[:-2] if len(blist) > 2 else blist

        def ffd(bl):
            tiles = []   # [used_partitions, [(r, c, idx, offset), ...]]
            for r, c, idx in bl:
                k = len(idx)
                for t in tiles:
                    if t[0] + k <= 128 and len(t[1]) < 5:
                        t[1].append((r, c, idx, t[0]))
                        t[0] += k
                        break
                else:
                    tiles.append([k, [(r, c, idx, 0)]])
            return tiles

        def balanced(bl, ntile):
            """least-loaded-first: balanced partition fill, <=5 buckets."""
            tiles = [[0, []] for _ in range(ntile)]
            for r, c, idx in bl:
                cand = [t for t in tiles if len(t[1]) < 5
                        and t[0] + len(idx) <= 128]
                if not cand:
                    return None
                t = min(cand, key=lambda t: t[0])
                t[1].append((r, c, idx, t[0]))
                t[0] += len(idx)
            return tiles

        tiles = balanced(main, (len(main) + 4) // 5) or ffd(main)
        if len(tiles) > (len(main) + 4) // 5:
            tiles = ffd(blist)            # runt split didn't help; fold back
            runt = []
        tl = [t[1] for t in tiles]
        tl.sort(key=lambda t: -len(t))   # most buckets (=units) first
        if runt:
            rt = []
            off = 0
            for r, c, idx in runt:
                rt.append((r, c, idx, off))
                off += len(idx)
            tl.append(rt)
        tiles_per_core.append(tl)
    return tiles_per_core


def _host_prep(np_inputs):
    """Build per-core device arrays:
      w12  [12, 256 + NT*128] fp16  (cols 0:256 = phi basis, then per-tile
                                     hi/lo quadratic coefficient columns)
      feat [128, NU*128]      fp16  (per-unit dense block feature weights)
    plus the placement map for host-side reassembly."""
    cx, cy, Aq, Bq, Cq, rx, ry, featw = _params(
        np_inputs["xyz_raw"], np_inputs["cholesky_raw"],
        np_inputs["features"], np_inputs["opacity"])
    tiles_per_core = _pack(cx, cy, rx, ry)

    NT = max(len(t) for t in tiles_per_core)
    U = []
    for t in range(NT):
        u = 1
        for core in range(NCORES):
            if t < len(tiles_per_core[core]):
                nb = len(tiles_per_core[core][t])
                u = max(u, -(-(nb * CH) // 128))
        U.append(u)
    NU = sum(U)

    # bucket-centered pixel coords: quarter-integers <= 56.25, exact in fp16
    xs = (np.arange(BK) + 0.5 - BK / 2).astype(np.float64)
    Yg, Xg = np.meshgrid(xs, xs, indexing="ij")
    phi6 = np.stack(
        [Xg * Xg, Yg * Yg, Xg * Yg, Xg, Yg, np.ones_like(Xg)], 0
    ).reshape(6, PXB)
    phi12 = np.concatenate([phi6, phi6], 0).astype(np.float16)

    w12 = np.zeros((NCORES, 12, 256 + NT * 128), dtype=np.float16)
    feat = np.zeros((NCORES, 128, NU * 128), dtype=np.float16)
    w12[:, :, 0:256] = phi12[None]
    placements = []   # per core: list of (tile, unit, pcol, m, ch0, r, c)
    for core in range(NCORES):
        place = []
        uoff = 0
        for t in range(NT):
            tl = tiles_per_core[core][t] if t < len(tiles_per_core[core]) else []
            fw16 = None
            cc = 0
            for r, c, idx, off in tl:
                k = len(idx)
                cxl = cx[idx] - c * BK - BK / 2
                cyl = cy[idx] - (core * NBR + r) * BK - BK / 2
                An, Bn, Cn = Aq[idx], Bq[idx], Cq[idx]
                W6 = np.stack(
                    [
                        An,
                        Cn,
                        Bn,
                        -(2.0 * An * cxl + Bn * cyl),
                        -(2.0 * Cn * cyl + Bn * cxl),
                        An * cxl * cxl + Cn * cyl * cyl + Bn * cxl * cyl,
                    ],
                    0,
                )
                W_hi = W6.astype(np.float16)
                W_lo = (W6 - W_hi.astype(np.float64)).astype(np.float16)
                base = 256 + t * 128 + off
                w12[core, :6, base:base + k] = W_hi
                w12[core, 6:, base:base + k] = W_lo
                fk = featw[idx].astype(np.float16)   # [k, 48]
                ch0 = 0
                while ch0 < CH:
                    unit, pcol = cc // 128, cc % 128
                    m = min(128 - pcol, CH - ch0)
                    feat[core, off:off + k,
                         (uoff + unit) * 128 + pcol:
                         (uoff + unit) * 128 + pcol + m] = fk[:, ch0:ch0 + m]
                    place.append((t, unit, pcol, m, ch0, r, c))
                    ch0 += m
                    cc += m
            uoff += U[t]
        placements.append(place)
    return w12, feat, NT, tuple(U), placements


def _build_program(NT, U, opool_bufs=5):
    import concourse.bacc as bacc
    import concourse.tile as tile
    import concourse.mybir as mybir

    NU = sum(U)
    toff = [0]
    for u in U:
        toff.append(toff[-1] + u * 256)

    nc = bacc.Bacc("TRN2", target_bir_lowering=False, debug=False,
                   num_devices=NCORES, num_swdge_queues=4)
    w12_ap = nc.dram_tensor("w12", [12, 256 + NT * 128], mybir.dt.float16,
                            kind="ExternalInput").ap()
    feat_ap = nc.dram_tensor("feat", [128, NU * 128], mybir.dt.float16,
                             kind="ExternalInput").ap()
    out_ap = nc.dram_tensor("out", [128, toff[-1]], mybir.dt.float16,
                            kind="ExternalOutput").ap()

    # first group is a single tile: its exp (and the first feature matmuls
    # and copies) start one sigma earlier, priming the whole pipeline
    groups = [(0, 1)] + [(g, min(g + 2, NT)) for g in range(1, NT, 2)]
    # feat arrives in 3 chunks timed to each stage's need: A = tile 0's units
    # (tiny, on SP right after w12 - lands ~1.6us before the first feature
    # matmul needs it), B1 = group 1's units (ACT HWDGE queue), B2 = rest
    nuA = U[0]
    nuB1 = sum(U[t] for t in range(1, groups[1][1])) if len(groups) > 1 else 0

    # copy plan: DVE handles early tiles inline (it is free while ACT runs
    # the exp chain); ACT takes the last NDEFER tiles, with those copies
    # emitted AFTER its final exp so the exp chain is never stretched.
    NDEFER = min(2, NT - 1)
    # (A SWDGE prepare/trigger path for the tail DMAs was tried: execution
    # is correct and ~0.5us faster, but the TimelineSim cost model never
    # posts the DMASW lane tick for triggered transfers, deadlocking the
    # timing metric - so output DMAs stay on plain DMACopy.)

    with tile.TileContext(nc) as tc:
        with ExitStack() as ctx:
            consts = ctx.enter_context(tc.tile_pool(name="consts", bufs=1))
            spool = ctx.enter_context(
                tc.tile_pool(name="sig", bufs=3, space="PSUM"))
            opool = ctx.enter_context(
                tc.tile_pool(name="acc", bufs=opool_bufs, space="PSUM"))
            wpool = ctx.enter_context(
                tc.tile_pool(name="accw", bufs=1, space="PSUM"))
            gpool = ctx.enter_context(tc.tile_pool(name="g", bufs=3))

            # PE HAM warmup: dummy matmuls on a zeroed SBUF tile while the
            # input DMAs are in flight, so the p-state ramp completes early.
            # (memset on Pool: DVE/ACT are the PSUM-evacuation bottleneck)
            dummy = consts.tile([12, 640], mybir.dt.float16)
            nc.gpsimd.memset(dummy, 0)
            for _ in range(2):
                # same variable name as the sigma tiles -> same pool tag
                psum_s = spool.tile([128, 512], mybir.dt.float32)
                nc.tensor.matmul(psum_s, dummy[:, 0:128], dummy[:, 128:640],
                                 start=True, stop=True)

            # inputs: w12 split so phi + first-group tiles land first (SP
            # HWDGE); feat chunk A on the ACT HWDGE queue (no waits, so it
            # cannot stall exp dispatch), rest on SP after the w12 chunks.
            w12_sb = consts.tile([12, 256 + NT * 128], mybir.dt.float16)
            nc.sync.dma_start(out=w12_sb, in_=w12_ap)
            phi_sb = w12_sb[:, 0:256]
            feat_sb = consts.tile([128, NU * 128], mybir.dt.float16)
            nc.sync.dma_start(out=feat_sb[:, :nuA * 128],
                              in_=feat_ap[:, :nuA * 128])
            nc.scalar.dma_start(out=feat_sb[:, nuA * 128:(nuA + nuB1) * 128],
                                in_=feat_ap[:, nuA * 128:(nuA + nuB1) * 128])
            nc.sync.dma_start(out=feat_sb[:, (nuA + nuB1) * 128:],
                              in_=feat_ap[:, (nuA + nuB1) * 128:])

            # single contiguous staging buffer: copies land in place, output
            # DMAs cover per-tile contiguous column ranges
            stage = consts.tile([128, toff[-1]], mybir.dt.float16)
            dma_q = [nc.sync, nc.gpsimd]   # two independent issue pipes

            def feat_matmuls(t, g, jj):
                psum_o = (wpool if U[t] > 2 else opool).tile(
                    [128, U[t] * 256], mybir.dt.float32)
                for u in range(U[t]):
                    nc.tensor.matmul(
                        psum_o[:, u * 256:(u + 1) * 256],
                        feat_sb[:, (toff[t] // 256 + u) * 128:
                                (toff[t] // 256 + u + 1) * 128],
                        g[:, jj * 256:(jj + 1) * 256],
                        start=True, stop=True)
                return psum_o

            def consume(t0, t1, g):
                """feature matmuls (+ inline DVE copy + DMA) per tile."""
                for j in range(t1 - t0):
                    t = t0 + j
                    psum_o = feat_matmuls(t, g, j)
                    if t < NT - NDEFER:
                        st = stage[:, toff[t]:toff[t + 1]]
                        nc.vector.tensor_copy(st, psum_o)
                        # SP issue pipe for early tiles (headroom, ~700ns
                        # less latency than a Pool-SWDGE gen); Pool for the
                        # second-to-last inline tile (a late SP DMACopy would
                        # block the tail DMAs on the in-order sequencer); the
                        # last inline tile rides the deferred merged DMA
                        if t < NT - NDEFER - 2:
                            nc.sync.dma_start(
                                out=out_ap[:, toff[t]:toff[t + 1]], in_=st)
                        elif t == NT - NDEFER - 2:
                            nc.gpsimd.dma_start(
                                out=out_ap[:, toff[t]:toff[t + 1]], in_=st)
                    else:
                        deferred.append((t, psum_o))

            # software pipeline: group g's sigma matmuls are emitted (and
            # run) BEFORE group g-1's feature matmuls on the PE stream, so
            # the serial exp chain on ACT never waits for PE.
            deferred = []
            prev = None
            for t0, t1 in groups:
                ntl = t1 - t0
                psum_s = spool.tile([128, ntl * 256], mybir.dt.float32)
                for j in range(ntl):
                    nc.tensor.matmul(
                        psum_s[:, j * 256:(j + 1) * 256],
                        w12_sb[:, 256 + (t0 + j) * 128:256 + (t0 + j + 1) * 128],
                        phi_sb, start=True, stop=True)
                g = gpool.tile([128, ntl * 256], mybir.dt.float16)
                nc.scalar.activation(
                    g, psum_s, mybir.ActivationFunctionType.Exp,
                    bias=0.0, scale=-1.0)
                if prev is not None:
                    consume(*prev)
                prev = (t0, t1, g)
            consume(*prev)

            # deferred tail tiles: ACT copies after its final exp; all but
            # the last share one merged SP DMA so the final DMA's issue slot
            # is free the moment the last copy lands
            for t, psum_o in deferred:
                st = stage[:, toff[t]:toff[t + 1]]
                nc.scalar.activation(
                    st, psum_o, mybir.ActivationFunctionType.Copy)
                if t == NT - 1:
                    # small runt DMA first (its data lands earliest), then
                    # the merged DMA covering the last inline tile + NT-2
                    nc.sync.dma_start(
                        out=out_ap[:, toff[t]:toff[t + 1]], in_=st)
                    t0d = max(NT - NDEFER - 1, 0)
                    if t0d < NT - 1:
                        nc.sync.dma_start(
                            out=out_ap[:, toff[t0d]:toff[NT - 1]],
                            in_=stage[:, toff[t0d]:toff[NT - 1]])
    nc.compile()
    return nc


def _get_program(NT, U):
    key = (NT, U)
    if key not in _cached:
        try:
            _cached[key] = _build_program(NT, U, opool_bufs=5)
        except ValueError:
            _cached[key] = _build_program(NT, U, opool_bufs=3)
    return _cached[key]


def kernel(xyz_raw, cholesky_raw, features, opacity):
    from concourse.bass_utils import run_bass_kernel_spmd

    np_inputs = {
        "xyz_raw": np.asarray(xyz_raw, dtype=np.float32),
        "cholesky_raw": np.asarray(cholesky_raw, dtype=np.float32),
        "features": np.asarray(features, dtype=np.float32),
        "opacity": np.asarray(opacity, dtype=np.float32),
    }
    w12, feat, NT, U, placements = _host_prep(np_inputs)
    nc = _get_program(NT, U)

    in_maps = [{"w12": w12[core], "feat": feat[core]}
               for core in range(NCORES)]
    res = run_bass_kernel_spmd(nc, in_maps, core_ids=list(range(NCORES)))

    toff = [0]
    for u in U:
        toff.append(toff[-1] + u * 256)
    out = np.zeros((CH, H, W), dtype=np.float32)
    for core in range(NCORES):
        strips = res.results[core]["out"].astype(np.float32)  # [128, TOT]
        for t, unit, pcol, m, ch0, r, c in placements[core]:
            sl = strips[pcol:pcol + m,
                        toff[t] + unit * 256:toff[t] + unit * 256 + 256]
            out[ch0:ch0 + m,
                (core * NBR + r) * BK:(core * NBR + r + 1) * BK,
                c * BK:(c + 1) * BK] += sl.reshape(m, BK, BK)
    return out.reshape(C, 3, H, W)


# revision 78
# speedup vs baseline: 1.0524x; 1.0524x over previous
"""Trainium2 Bass kernel for nn_GaussianBasis (2D gaussian-splat sum rasterizer).

Math: out[c,d,h,w] = sum_n opacity_n * exp(-sigma_n(h,w)) * features[c,n,d]
where sigma is a per-gaussian quadratic form in pixel coords.

Strategy:
  - Gaussians have tiny support; bin them host-side into 16x16-pixel buckets
    with a sigma <= SIG_CUT cutoff ellipse (exp(-8) ~ 3.4e-4; dropped tail
    contributions stay far below the 2e-2 relative tolerance vs absmax ~2.7).
  - Each core owns a 32-row band = 2x16 buckets. Buckets are packed into
    128-partition "tiles" (one gaussian = one partition row, sum k <= 128,
    <= 5 buckets so each tile needs exactly 2 channel-units; the 2 smallest
    buckets form a final 1-unit "runt" tile so the tail transfer is tiny).
    sigma over a tile is ONE K=12 fp16 matmul against the shared
    bucket-centered phi = [x^2,y^2,xy,x,y,1] basis (hi/lo fp16 coefficient
    split for fp32-grade accuracy) -> [128 gauss, 256 px] PSUM. Packing cuts
    tiles from 16 to 7, cutting the serial exp chain ~2.3x.
  - ACT computes g = exp(-sigma) PSUM->SBUF fp16 in 2-tile groups; the first
    group is a single tile so the downstream pipeline primes one sigma
    earlier. Group g+1's sigma matmuls are emitted before group g's feature
    matmuls (software pipelining) so the exp chain never waits on PE.
  - Feature einsum: per tile, the buckets' 48 output channels form one dense
    column stream chopped into 128-column "units" (a bucket's channels may
    split across units): one fp16 matmul per unit with a [128,128] block
    weight -> PSUM [128, 256] with EVERY partition carrying payload - the
    information-theoretic floor for PSUM-evacuation volume (13 units/core).
  - PSUM is only readable by ACT/DVE (no DMA, no Pool). Per-tile fp32->fp16
    copies into one contiguous SBUF staging buffer: DVE inline for early
    tiles (it idles while ACT runs exps), ACT for the last NDEFER tiles,
    emitted after its final exp so the exp chain is never stretched.
  - Per-tile output DMAs (fp16, runs >= 512B: full bandwidth) overlap
    compute, alternating the SP-HWDGE and Pool-SWDGE issue pipes; the
    deferred tiles share one merged DMA plus a small final one. Host
    upcasts and reassembles the strips into [C,3,H,W].
  - No collectives: pixel-sharding keeps outputs disjoint.
"""

import sys

sys.path.insert(0, "/opt/trn_rl_repo")

import numpy as np
from contextlib import ExitStack

N, C, H, W = 2048, 16, 256, 256
NCORES = 8
BK = 16                      # bucket edge in pixels
PXB = BK * BK                # 256 px per bucket
NBR, NBC = (H // NCORES) // BK, W // BK   # 2 bucket-rows, 16 cols per core
CH = C * 3                   # 48 output channels
SIG_CUT = 8.0                # exp(-8) ~ 3.4e-4: negligible vs tolerance

_cached = {}


def _params(xyz_raw, cholesky_raw, features, opacity):
    """Per-gaussian params (fp64 host): centers, quadratic coeffs, cutoff
    radii, opacity-folded features."""
    xy = np.tanh(xyz_raw.astype(np.float64))
    cx = 0.5 * (xy[:, 0] + 1.0) * W
    cy = 0.5 * (xy[:, 1] + 1.0) * H
    chol = cholesky_raw.astype(np.float64) + np.array([0.5, 0.0, 0.5])
    l1, l2, l3 = chol[:, 0], chol[:, 1], chol[:, 2]
    a = l1 * l1
    b = l1 * l2
    c = l2 * l2 + l3 * l3
    det = a * c - b * b
    Aq, Bq, Cq = 0.5 * (c / det), -b / det, 0.5 * (a / det)
    rx = np.sqrt(2.0 * SIG_CUT * a) + 1.0
    ry = np.sqrt(2.0 * SIG_CUT * c) + 1.0
    featw = features.astype(np.float64) * opacity[:, 0][None, :, None]
    featw = np.transpose(featw, (1, 0, 2)).reshape(N, CH)
    return cx, cy, Aq, Bq, Cq, rx, ry, featw


def _pack(cx, cy, rx, ry):
    """Bin gaussians into per-core 16x16 buckets; first-fit-decreasing pack
    whole buckets into 128-partition tiles (oversized buckets split).
    Returns tiles[core] = list of tiles; tile = [(r, c, idx, offset), ...]
    sorted so tiles with more channel-units come first (aligns padding)."""
    h_lo = np.floor(cy - ry).astype(int)
    h_hi = np.ceil(cy + ry).astype(int)
    w_lo = np.floor(cx - rx).astype(int)
    w_hi = np.ceil(cx + rx).astype(int)
    nrow, ncol = H // BK, W // BK
    buckets = [[[] for _ in range(ncol)] for _ in range(nrow)]
    for n in range(N):
        for bh in range(max(0, h_lo[n] // BK), min(nrow, h_hi[n] // BK + 1)):
            for bw in range(max(0, w_lo[n] // BK), min(ncol, w_hi[n] // BK + 1)):
                buckets[bh][bw].append(n)

    tiles_per_core = []
    for core in range(NCORES):
        blist = []
        for r in range(NBR):
            for c in range(NBC):
                ns = np.asarray(buckets[core * NBR + r][c], dtype=int)
                for s in range(0, len(ns), 128):
                    blist.append((r, c, ns[s:s + 128]))
        blist.sort(key=lambda b: -len(b[2]))
        # cap buckets/tile at 5 so every tile needs exactly 2 channel-units
        # (48*5=240 <= 256 psum cols): uniform tiles, no oversized PSUM.
        # The 2 smallest buckets are reserved as a 1-unit "runt" tile that
        # goes LAST: the tail copy+DMA is then half-sized.
        runt = blist[-2:] if len(blist) > 2 else []
        main = blist[:-2] if len(blist) > 2 else blist

        def ffd(bl):
            tiles = []   # [used_partitions, [(r, c, idx, offset), ...]]
            for r, c, idx in bl:
                k = len(idx)
                for t in tiles:
                    if t[0] + k <= 128 and len(t[1]) < 5:
                        t[1].append((r, c, idx, t[0]))
                        t[0] += k
                        break
                else:
                    tiles.append([k, [(r, c, idx, 0)]])
            return tiles

        def balanced(bl, ntile):
            """least-loaded-first: balanced partition fill, <=5 buckets."""
            tiles = [[0, []] for _ in range(ntile)]
            for r, c, idx in bl:
                cand = [t for t in tiles if len(t[1]) < 5
                        and t[0] + len(idx) <= 128]
                if not cand:
                    return None
                t = min(cand, key=lambda t: t[0])
                t[1].append((r, c, idx, t[0]))
                t[0] += len(idx)
            return tiles

        tiles = balanced(main, (len(main) + 4) // 5) or ffd(main)
        if len(tiles) > (len(main) + 4) // 5:
            tiles = ffd(blist)            # runt split didn't help; fold back
            runt = []
        tl = [t[1] for t in tiles]
        tl.sort(key=lambda t: -len(t))   # most buckets (=units) first
        if runt:
            rt = []
            off = 0
            for r, c, idx in runt:
                rt.append((r, c, idx, off))
                off += len(idx)
            tl.append(rt)
        tiles_per_core.append(tl)
    return tiles_per_core


def _host_prep(np_inputs):
    """Build per-core device arrays:
      w12  [12, 256 + NT*128] fp16  (cols 0:256 = phi basis, then per-tile
                                     hi/lo quadratic coefficient columns)
      feat [128, NU*128]      fp16  (per-unit dense block feature weights)
    plus the placement map for host-side reassembly."""
    cx, cy, Aq, Bq, Cq, rx, ry, featw = _params(
        np_inputs["xyz_raw"], np_inputs["cholesky_raw"],
        np_inputs["features"], np_inputs["opacity"])
    tiles_per_core = _pack(cx, cy, rx, ry)

    NT = max(len(t) for t in tiles_per_core)
    U = []
    for t in range(NT):
        u = 1
        for core in range(NCORES):
            if t < len(tiles_per_core[core]):
                nb = len(tiles_per_core[core][t])
                u = max(u, -(-(nb * CH) // 128))
        U.append(u)
    NU = sum(U)

    # bucket-centered pixel coords: quarter-integers <= 56.25, exact in fp16
    xs = (np.arange(BK) + 0.5 - BK / 2).astype(np.float64)
    Yg, Xg = np.meshgrid(xs, xs, indexing="ij")
    phi6 = np.stack(
        [Xg * Xg, Yg * Yg, Xg * Yg, Xg, Yg, np.ones_like(Xg)], 0
    ).reshape(6, PXB)
    phi12 = np.concatenate([phi6, phi6], 0).astype(np.float16)

    w12 = np.zeros((NCORES, 12, 256 + NT * 128), dtype=np.float16)
    feat = np.zeros((NCORES, 128, NU * 128), dtype=np.float16)
    w12[:, :, 0:256] = phi12[None]
    placements = []   # per core: list of (tile, unit, pcol, m, ch0, r, c)
    for core in range(NCORES):
        place = []
        uoff = 0
        for t in range(NT):
            tl = tiles_per_core[core][t] if t < len(tiles_per_core[core]) else []
            fw16 = None
            cc = 0
            for r, c, idx, off in tl:
                k = len(idx)
                cxl = cx[idx] - c * BK - BK / 2
                cyl = cy[idx] - (core * NBR + r) * BK - BK / 2
                An, Bn, Cn = Aq[idx], Bq[idx], Cq[idx]
                W6 = np.stack(
                    [
                        An,
                        Cn,
                        Bn,
                        -(2.0 * An * cxl + Bn * cyl),
                        -(2.0 * Cn * cyl + Bn * cxl),
                        An * cxl * cxl + Cn * cyl * cyl + Bn * cxl * cyl,
                    ],
                    0,
                )
                W_hi = W6.astype(np.float16)
                W_lo = (W6 - W_hi.astype(np.float64)).astype(np.float16)
                base = 256 + t * 128 + off
                w12[core, :6, base:base + k] = W_hi
                w12[core, 6:, base:base + k] = W_lo
                fk = featw[idx].astype(np.float16)   # [k, 48]
                ch0 = 0
                while ch0 < CH:
                    unit, pcol = cc // 128, cc % 128
                    m = min(128 - pcol, CH - ch0)
                    feat[core, off:off + k,
                         (uoff + unit) * 128 + pcol:
                         (uoff + unit) * 128 + pcol + m] = fk[:, ch0:ch0 + m]
                    place.append((t, unit, pcol, m, ch0, r, c))
                    ch0 += m
                    cc += m
            uoff += U[t]
        placements.append(place)
    return w12, feat, NT, tuple(U), placements


def _build_program(NT, U, opool_bufs=5):
    import concourse.bacc as bacc
    import concourse.tile as tile
    import concourse.mybir as mybir

    NU = sum(U)
    toff = [0]
    for u in U:
        toff.append(toff[-1] + u * 256)

    nc = bacc.Bacc("TRN2", target_bir_lowering=False, debug=False,
                   num_devices=NCORES, num_swdge_queues=4)
    w12_ap = nc.dram_tensor("w12", [12, 256 + NT * 128], mybir.dt.float16,
                            kind="ExternalInput").ap()
    feat_ap = nc.dram_tensor("feat", [128, NU * 128], mybir.dt.float16,
                             kind="ExternalInput").ap()
    out_ap = nc.dram_tensor("out", [128, toff[-1]], mybir.dt.float16,
                            kind="ExternalOutput").ap()

    # first group is a single tile: its exp (and the first feature matmuls
    # and copies) start one sigma earlier, priming the whole pipeline
    groups = [(0, 1)] + [(g, min(g + 2, NT)) for g in range(1, NT, 2)]
    nuA = sum(U[t] for t in range(0, groups[min(1, len(groups) - 1)][1]))

    # copy plan: DVE handles early tiles inline (it is free while ACT runs
    # the exp chain); ACT takes the last NDEFER tiles, with those copies
    # emitted AFTER its final exp so the exp chain is never stretched.
    NDEFER = min(3, NT - 1)
    # (A SWDGE prepare/trigger path for the tail DMAs was tried: execution
    # is correct and ~0.5us faster, but the TimelineSim cost model never
    # posts the DMASW lane tick for triggered transfers, deadlocking the
    # timing metric - so output DMAs stay on plain DMACopy.)

    with tile.TileContext(nc) as tc:
        with ExitStack() as ctx:
            consts = ctx.enter_context(tc.tile_pool(name="consts", bufs=1))
            spool = ctx.enter_context(
                tc.tile_pool(name="sig", bufs=3, space="PSUM"))
            opool = ctx.enter_context(
                tc.tile_pool(name="acc", bufs=opool_bufs, space="PSUM"))
            wpool = ctx.enter_context(
                tc.tile_pool(name="accw", bufs=1, space="PSUM"))
            gpool = ctx.enter_context(tc.tile_pool(name="g", bufs=3))

            # PE HAM warmup: dummy matmuls on a zeroed SBUF tile while the
            # input DMAs are in flight, so the p-state ramp completes early.
            # (memset on Pool: DVE/ACT are the PSUM-evacuation bottleneck)
            dummy = consts.tile([12, 640], mybir.dt.float16)
            nc.gpsimd.memset(dummy, 0)
            for _ in range(2):
                # same variable name as the sigma tiles -> same pool tag
                psum_s = spool.tile([128, 512], mybir.dt.float32)
                nc.tensor.matmul(psum_s, dummy[:, 0:128], dummy[:, 128:640],
                                 start=True, stop=True)

            # inputs: w12 split so phi + first-group tiles land first (SP
            # HWDGE); feat chunk A on the ACT HWDGE queue (no waits, so it
            # cannot stall exp dispatch), rest on SP after the w12 chunks.
            w12_sb = consts.tile([12, 256 + NT * 128], mybir.dt.float16)
            nc.sync.dma_start(out=w12_sb, in_=w12_ap)
            phi_sb = w12_sb[:, 0:256]
            feat_sb = consts.tile([128, NU * 128], mybir.dt.float16)
            nc.scalar.dma_start(out=feat_sb[:, :nuA * 128],
                                in_=feat_ap[:, :nuA * 128])
            nc.sync.dma_start(out=feat_sb[:, nuA * 128:],
                              in_=feat_ap[:, nuA * 128:])

            # single contiguous staging buffer: copies land in place, output
            # DMAs cover per-tile contiguous column ranges
            stage = consts.tile([128, toff[-1]], mybir.dt.float16)
            dma_q = [nc.sync, nc.gpsimd]   # two independent issue pipes

            def feat_matmuls(t, g, jj):
                psum_o = (wpool if U[t] > 2 else opool).tile(
                    [128, U[t] * 256], mybir.dt.float32)
                for u in range(U[t]):
                    nc.tensor.matmul(
                        psum_o[:, u * 256:(u + 1) * 256],
                        feat_sb[:, (toff[t] // 256 + u) * 128:
                                (toff[t] // 256 + u + 1) * 128],
                        g[:, jj * 256:(jj + 1) * 256],
                        start=True, stop=True)
                return psum_o

            def consume(t0, t1, g):
                """feature matmuls (+ inline DVE copy + DMA) per tile."""
                for j in range(t1 - t0):
                    t = t0 + j
                    psum_o = feat_matmuls(t, g, j)
                    if t < NT - NDEFER:
                        st = stage[:, toff[t]:toff[t + 1]]
                        nc.vector.tensor_copy(st, psum_o)
                        # SP issue pipe for early tiles (it has headroom and
                        # ~700ns less latency than a Pool-SWDGE gen); Pool
                        # only for the last inline tile, whose late DMACopy
                        # would otherwise block the deferred merged DMA on
                        # the in-order SP sequencer
                        q = nc.gpsimd if t == NT - NDEFER - 1 else nc.sync
                        q.dma_start(
                            out=out_ap[:, toff[t]:toff[t + 1]], in_=st)
                    else:
                        deferred.append((t, psum_o))

            # software pipeline: group g's sigma matmuls are emitted (and
            # run) BEFORE group g-1's feature matmuls on the PE stream, so
            # the serial exp chain on ACT never waits for PE.
            deferred = []
            prev = None
            for t0, t1 in groups:
                ntl = t1 - t0
                psum_s = spool.tile([128, ntl * 256], mybir.dt.float32)
                for j in range(ntl):
                    nc.tensor.matmul(
                        psum_s[:, j * 256:(j + 1) * 256],
                        w12_sb[:, 256 + (t0 + j) * 128:256 + (t0 + j + 1) * 128],
                        phi_sb, start=True, stop=True)
                g = gpool.tile([128, ntl * 256], mybir.dt.float16)
                nc.scalar.activation(
                    g, psum_s, mybir.ActivationFunctionType.Exp,
                    bias=0.0, scale=-1.0)
                if prev is not None:
                    consume(*prev)
                prev = (t0, t1, g)
            consume(*prev)

            # deferred tail tiles: ACT copies after its final exp; all but
            # the last share one merged SP DMA so the final DMA's issue slot
            # is free the moment the last copy lands
            for t, psum_o in deferred:
                st = stage[:, toff[t]:toff[t + 1]]
                nc.scalar.activation(
                    st, psum_o, mybir.ActivationFunctionType.Copy)
                if t == NT - 2 and len(deferred) > 1:
                    t0d = deferred[0][0]
                    nc.sync.dma_start(
                        out=out_ap[:, toff[t0d]:toff[NT - 1]],
                        in_=stage[:, toff[t0d]:toff[NT - 1]])
                elif t == NT - 1:
                    nc.sync.dma_start(
                        out=out_ap[:, toff[t]:toff[t + 1]], in_=st)
    nc.compile()
    return nc


def _get_program(NT, U):
    key = (NT, U)
    if key not in _cached:
        try:
            _cached[key] = _build_program(NT, U, opool_bufs=5)
        except ValueError:
            _cached[key] = _build_program(NT, U, opool_bufs=3)
    return _cached[key]


def kernel(xyz_raw, cholesky_raw, features, opacity):
    from concourse.bass_utils import run_bass_kernel_spmd

    np_inputs = {
        "xyz_raw": np.asarray(xyz_raw, dtype=np.float32),
        "cholesky_raw": np.asarray(cholesky_raw, dtype=np.float32),
        "features": np.asarray(features, dtype=np.float32),
        "opacity": np.asarray(opacity, dtype=np.float32),
    }
    w12, feat, NT, U, placements = _host_prep(np_inputs)
    nc = _get_program(NT, U)

    in_maps = [{"w12": w12[core], "feat": feat[core]}
               for core in range(NCORES)]
    res = run_bass_kernel_spmd(nc, in_maps, core_ids=list(range(NCORES)))

    toff = [0]
    for u in U:
        toff.append(toff[-1] + u * 256)
    out = np.zeros((CH, H, W), dtype=np.float32)
    for core in range(NCORES):
        strips = res.results[core]["out"].astype(np.float32)  # [128, TOT]
        for t, unit, pcol, m, ch0, r, c in placements[core]:
            sl = strips[pcol:pcol + m,
                        toff[t] + unit * 256:toff[t] + unit * 256 + 256]
            out[ch0:ch0 + m,
                (core * NBR + r) * BK:(core * NBR + r + 1) * BK,
                c * BK:(c + 1) * BK] += sl.reshape(m, BK, BK)
    return out.reshape(C, 3, H, W)
